# revision 5
# baseline (speedup 1.0000x reference)
"""GNN message-passing (HCA) kernel for 8 TRN2 NeuronCores.

Strategy: node-shard 50000 nodes as 8 x 6250. Per core (all compute bf16,
f32 accumulation in PSUM):
  phase A: feature-major pre-MLP  h1=tanh(x@W1+b1), h2=tanh(h1@W2+b2),
           m=h2@Wmp+bmp on TensorE; PE-transpose m to node-major rows;
           spill h2^T to HBM; write m rows to per-chunk bounce buffers.
  AllGather (5 row-chunks) -> m_full [50000,1024] bf16 in every core's HBM
           (rows permuted chunk-major; host remaps gather indices).
  phase B: per dst-tile (128 owned dst nodes): dma_gather the in-edge
           source rows of m_full (edges sorted by dst tile, split into
           low/high halves for int16 indexing, padded to 128-blocks), then
           segment-sum via one-hot matmuls accumulating in PSUM; tanh -> z;
           PE-transpose z; post-MLP out=softplus([z,h2]@Wpost+bpost) fused,
           DMA out.
Host-side numpy does only index/layout prep (edge sorting, one-hot
construction, dtype casts); all FLOPs on the value path run on device.
"""
import sys
if '/opt/trn_rl_repo' not in sys.path:
    sys.path.insert(0, '/opt/trn_rl_repo')
import numpy as np
import ml_dtypes

bf16 = ml_dtypes.bfloat16

N, E, F, H = 50000, 400000, 16, 1024
NCORES = 8
OWN = N // NCORES            # 6250
NT = (OWN + 127) // 128      # 49 dst tiles per core
LOW = 25600                  # rows >= LOW live in m_fullB (all fit int16)
# AllGather row chunks per core (128-aligned first chunk; 3200 + 3050 = 6250)
CHSZ = [3200, 3050]
KST = [0, 3200]
NCHUNK = [512] * 12 + [106]                # phase-A node chunks (sum 6250)

_cache = {}


def _remap_rows(g):
    """global node id -> unified row id: [0,25600) in m_fullA,
    25600+ [0,24400) in m_fullB."""
    c = g // OWN
    j = g % OWN
    return np.where(j < 3200, c * 3200 + j,
                    LOW + c * 3050 + (j - 3200))


def _pack_idx(idx):
    """int32 indices -> int16 [128, n/16]: idx i at [i%16, i//16], replicated."""
    n = len(idx)
    a = np.asarray(idx, np.int16).reshape(n // 16, 16).T
    return np.ascontiguousarray(np.tile(a, (8, 1)))


def _prep_edges(edge_src, edge_dst):
    """Returns (NL, NH, per_core list of (idx_packed, onehot))."""
    src = np.asarray(edge_src, np.int64)
    dst = np.asarray(edge_dst, np.int64)
    owner = dst // OWN
    jd = dst % OWN
    tile = jd // 128
    col = jd % 128
    mrow = _remap_rows(src)
    hi = (mrow >= LOW).astype(np.int64)

    cnt = np.zeros((NCORES, NT, 2), np.int64)
    np.add.at(cnt, (owner, tile, hi), 1)
    nb = (cnt + 127) // 128
    NL = nb[:, :, 0].max(axis=0)
    NH = nb[:, :, 1].max(axis=0)
    NL = np.maximum(NL, 1)          # >=1 block per tile so PSUM gets start=True
    nbt = np.stack([NL, NH], axis=1)          # [NT, 2]
    gsize = nbt * 128                         # padded group sizes
    goff = np.concatenate([[0], np.cumsum(gsize.reshape(-1))])[:-1].reshape(NT, 2)
    tot = int(gsize.sum())
    totb = tot // 128

    per_core = []
    for c in range(NCORES):
        sel = owner == c
        t_c, h_c, col_c, mrow_c = tile[sel], hi[sel], col[sel], mrow[sel]
        key = t_c * 2 + h_c
        order = np.argsort(key, kind='stable')
        t_s, h_s, col_s, mrow_s = (t_c[order], h_c[order], col_c[order],
                                   mrow_c[order])
        ks = key[order]
        # position within group
        grp_cnt = np.bincount(ks, minlength=NT * 2)
        starts = np.concatenate([[0], np.cumsum(grp_cnt)])[:-1]
        pos = np.arange(len(ks)) - np.repeat(starts, grp_cnt)
        flat = goff[t_s, h_s] + pos
        idx_flat = np.zeros(tot, np.int32)
        idx_flat[flat] = np.where(h_s == 1, mrow_s - LOW, mrow_s)
        oh = np.zeros((totb, 128, 128), bf16)
        oh[flat // 128, flat % 128, col_s] = bf16(1.0)
        per_core.append((_pack_idx(idx_flat), oh))
    return NL.astype(int), NH.astype(int), goff, per_core


def _build(NL, NH, goff):
    import concourse.bass as bass
    import concourse.mybir as mybir
    import concourse.tile as tile
    import concourse.bacc as bacc

    dt = mybir.dt
    TOT = int((np.asarray(NL) + np.asarray(NH)).sum() * 128)
    TOTB = TOT // 128

    nc = bacc.Bacc("TRN2", target_bir_lowering=False, debug=False,
                   num_devices=NCORES)
    xT_in = nc.dram_tensor("xT_in", [F, OWN], dt.bfloat16, kind="ExternalInput")
    W1_in = nc.dram_tensor("W1_in", [F, H], dt.bfloat16, kind="ExternalInput")
    W2_in = nc.dram_tensor("W2_in", [H, H], dt.bfloat16, kind="ExternalInput")
    Wmp_in = nc.dram_tensor("Wmp_in", [H, H], dt.bfloat16, kind="ExternalInput")
    Wpost_in = nc.dram_tensor("Wpost_in", [2 * H, F], dt.bfloat16, kind="ExternalInput")
    b1_in = nc.dram_tensor("b1_in", [128, 8], dt.float32, kind="ExternalInput")
    b2_in = nc.dram_tensor("b2_in", [128, 8], dt.float32, kind="ExternalInput")
    bmp_in = nc.dram_tensor("bmp_in", [128, 8], dt.float32, kind="ExternalInput")
    bpost_in = nc.dram_tensor("bpost_in", [1, F], dt.bfloat16, kind="ExternalInput")
    ident_in = nc.dram_tensor("ident_in", [128, 128], dt.bfloat16, kind="ExternalInput")
    idx_in = nc.dram_tensor("idx_in", [128, TOT // 16], dt.int16, kind="ExternalInput")
    oh_in = nc.dram_tensor("oh_in", [TOTB, 128, 128], dt.bfloat16, kind="ExternalInput")
    out_ext = nc.dram_tensor("out", [OWN, F], dt.float32, kind="ExternalOutput")

    with tile.TileContext(nc) as tc:
        with (
            tc.tile_pool(name="dram", bufs=1, space="DRAM") as dram,
            tc.tile_pool(name="wpool", bufs=1) as wpool,
            tc.tile_pool(name="cpool", bufs=1) as cpool,
        ):
            h2T_spill = dram.tile([H, OWN], dt.bfloat16)
            m_fullA = dram.tile([NCORES * CHSZ[0], H], dt.bfloat16,
                                addr_space="Shared")
            m_fullB = dram.tile([NCORES * CHSZ[1], H], dt.bfloat16,
                                addr_space="Shared")
            bounces = [dram.tile([CHSZ[k], H], dt.bfloat16, name=f"bounce{k}")
                       for k in range(len(CHSZ))]

            # resident weights / constants
            W1_sb = wpool.tile([F, H], dt.bfloat16)
            nc.sync.dma_start(W1_sb[:], W1_in.ap())
            W2_sb = wpool.tile([128, 8, H], dt.bfloat16)
            Wmp_sb = wpool.tile([128, 8, H], dt.bfloat16)
            for k in range(8):
                nc.sync.dma_start(W2_sb[:, k, :], W2_in.ap()[k * 128:(k + 1) * 128, :])
                nc.sync.dma_start(Wmp_sb[:, k, :], Wmp_in.ap()[k * 128:(k + 1) * 128, :])
            Wpost_sb = wpool.tile([128, 16, F], dt.bfloat16)
            for k in range(16):
                nc.sync.dma_start(Wpost_sb[:, k, :], Wpost_in.ap()[k * 128:(k + 1) * 128, :])
            xT_sb = wpool.tile([F, OWN], dt.bfloat16)
            nc.sync.dma_start(xT_sb[:], xT_in.ap())
            b1_sb = cpool.tile([128, 8], dt.float32)
            b2_sb = cpool.tile([128, 8], dt.float32)
            bmp_sb = cpool.tile([128, 8], dt.float32)
            nc.sync.dma_start(b1_sb[:], b1_in.ap())
            nc.sync.dma_start(b2_sb[:], b2_in.ap())
            nc.sync.dma_start(bmp_sb[:], bmp_in.ap())
            bpost_sb = cpool.tile([1, F], dt.bfloat16)
            nc.sync.dma_start(bpost_sb[:], bpost_in.ap())
            ident_sb = cpool.tile([128, 128], dt.bfloat16)
            nc.sync.dma_start(ident_sb[:], ident_in.ap())
            ones_sb = cpool.tile([1, 128], dt.bfloat16)
            nc.vector.memset(ones_sb[:], 1.0)
            sbatch = cpool.tile([128, NT, F], dt.float32)
            nc.vector.memset(sbatch[:], 0.5)
            idx_sb = cpool.tile([128, TOT // 16], dt.int16)
            nc.sync.dma_start(idx_sb[:], idx_in.ap())

            # ---------------- phase A ----------------
            with (
                tc.tile_pool(name="h1p", bufs=2) as h1p,
                tc.tile_pool(name="h2p", bufs=10) as h2p,
                tc.tile_pool(name="mTp", bufs=3) as mTp,
                tc.tile_pool(name="mnp", bufs=6) as mnp,
                tc.tile_pool(name="psA", bufs=2, space="PSUM") as psA,
                tc.tile_pool(name="psAt", bufs=2, space="PSUM") as psAt,
            ):
                n0 = 0
                for nn in NCHUNK:
                    h1 = h1p.tile([128, 8, 512], dt.bfloat16, name="h1")
                    for ft in range(8):
                        ps = psA.tile([128, 512], dt.float32, name="psa")
                        nc.tensor.matmul(ps[:, :nn], W1_sb[:, ft * 128:(ft + 1) * 128],
                                         xT_sb[:, n0:n0 + nn], start=True, stop=True)
                        nc.scalar.activation(h1[:, ft, :nn], ps[:, :nn],
                                             mybir.ActivationFunctionType.Tanh,
                                             bias=b1_sb[:, ft:ft + 1])
                    h2l = []
                    for ft in range(8):
                        ps = psA.tile([128, 512], dt.float32, name="psa")
                        for k in range(8):
                            nc.tensor.matmul(ps[:, :nn],
                                             W2_sb[:, k, ft * 128:(ft + 1) * 128],
                                             h1[:, k, :nn],
                                             start=(k == 0), stop=(k == 7))
                        h2t = h2p.tile([128, 512], dt.bfloat16, name="h2t")
                        nc.scalar.activation(h2t[:, :nn], ps[:, :nn],
                                             mybir.ActivationFunctionType.Tanh,
                                             bias=b2_sb[:, ft:ft + 1])
                        nc.sync.dma_start(
                            h2T_spill[ft * 128:(ft + 1) * 128, n0:n0 + nn],
                            h2t[:, :nn])
                        h2l.append(h2t)
                    nsub = (nn + 127) // 128
                    mns = [mnp.tile([128, H], dt.bfloat16, name="mn") for _ in range(nsub)]
                    for ft in range(8):
                        ps = psA.tile([128, 512], dt.float32, name="psa")
                        for k in range(8):
                            nc.tensor.matmul(ps[:, :nn],
                                             Wmp_sb[:, k, ft * 128:(ft + 1) * 128],
                                             h2l[k][:, :nn],
                                             start=(k == 0), stop=(k == 7))
                        mT = mTp.tile([128, 512], dt.bfloat16, name="mT")
                        nc.vector.tensor_scalar_add(mT[:, :nn], ps[:, :nn],
                                                    bmp_sb[:, ft:ft + 1])
                        for s in range(nsub):
                            ns = min(128, nn - 128 * s)
                            tp = psAt.tile([128, 128], dt.bfloat16, name="tp")
                            nc.tensor.transpose(tp[:ns, :], mT[:, s * 128:s * 128 + ns],
                                                ident_sb[:])
                            nc.vector.tensor_copy(
                                mns[s][:ns, ft * 128:(ft + 1) * 128], tp[:ns, :])
                    for s in range(nsub):
                        j0 = n0 + 128 * s
                        ns = min(128, nn - 128 * s)
                        k = 0 if j0 < 3200 else 1
                        nc.sync.dma_start(
                            bounces[k][j0 - KST[k]:j0 - KST[k] + ns, :],
                            mns[s][:ns, :])
                    n0 += nn

            # ---------------- AllGather ----------------
            for k, mf in enumerate((m_fullA, m_fullB)):
                nc.gpsimd.collective_compute(
                    "AllGather", mybir.AluOpType.bypass,
                    replica_groups=[list(range(NCORES))],
                    ins=[bounces[k].opt()],
                    outs=[mf.opt()],
                )

            # ---------------- phase B ----------------
            with (
                tc.tile_pool(name="Gp", bufs=2) as Gp,
                tc.tile_pool(name="ohp", bufs=6) as ohp,
                tc.tile_pool(name="zp", bufs=2) as zp,
                tc.tile_pool(name="zTp", bufs=16) as zTp,
                tc.tile_pool(name="h2r", bufs=2) as h2r,
                tc.tile_pool(name="outp", bufs=2) as outp,
                tc.tile_pool(name="psAcc", bufs=4, space="PSUM") as psAcc,
                tc.tile_pool(name="psBt", bufs=2, space="PSUM") as psBt,
                tc.tile_pool(name="psPost", bufs=2, space="PSUM") as psPost,
            ):
                obi = 0
                for t in range(NT):
                    nt = min(128, OWN - 128 * t)
                    nbl, nbh = int(NL[t]), int(NH[t])
                    GL = Gp.tile([128, nbl, H], dt.bfloat16, name="GL", tag="G")
                    nc.gpsimd.dma_gather(
                        GL[:], m_fullA[:, :],
                        idx_sb[:, goff[t, 0] // 16:(goff[t, 0] + nbl * 128) // 16],
                        nbl * 128, nbl * 128, elem_size=H)
                    if nbh:
                        GH = Gp.tile([128, nbh, H], dt.bfloat16, name="GH", tag="G")
                        nc.gpsimd.dma_gather(
                            GH[:], m_fullB[:, :],
                            idx_sb[:, goff[t, 1] // 16:(goff[t, 1] + nbh * 128) // 16],
                            nbh * 128, nbh * 128, elem_size=H)
                    acc0 = psAcc.tile([128, 512], dt.float32, name="acc0", tag="acc")
                    acc1 = psAcc.tile([128, 512], dt.float32, name="acc1", tag="acc")
                    blocks = [(GL, b) for b in range(nbl)]
                    if nbh:
                        blocks += [(GH, b) for b in range(nbh)]
                    nb = len(blocks)
                    for bi, (g, b) in enumerate(blocks):
                        oh = ohp.tile([128, 128], dt.bfloat16, name="oh")
                        nc.sync.dma_start(oh[:], oh_in.ap()[obi])
                        obi += 1
                        nc.tensor.matmul(acc0[:], oh[:], g[:, b, 0:512],
                                         start=(bi == 0), stop=(bi == nb - 1))
                        nc.tensor.matmul(acc1[:], oh[:], g[:, b, 512:1024],
                                         start=(bi == 0), stop=(bi == nb - 1))
                    z = zp.tile([128, H], dt.bfloat16, name="z")
                    nc.scalar.activation(z[:, 0:512], acc0[:],
                                         mybir.ActivationFunctionType.Tanh)
                    nc.scalar.activation(z[:, 512:1024], acc1[:],
                                         mybir.ActivationFunctionType.Tanh)
                    zTs = []
                    for f in range(8):
                        tp = psBt.tile([128, 128], dt.bfloat16, name="tpz")
                        nc.tensor.transpose(tp[:], z[:, f * 128:(f + 1) * 128],
                                            ident_sb[:])
                        zT = zTp.tile([128, 128], dt.bfloat16, name="zT")
                        nc.vector.tensor_copy(zT[:], tp[:])
                        zTs.append(zT)
                    h2t = h2r.tile([128, 8, 128], dt.bfloat16, name="h2rt")
                    for f in range(8):
                        nc.sync.dma_start(
                            h2t[:, f, :nt],
                            h2T_spill[f * 128:(f + 1) * 128, t * 128:t * 128 + nt])
                    pp = psPost.tile([128, F], dt.float32, name="pp")
                    for kt in range(16):
                        lhsT = zTs[kt][:, :nt] if kt < 8 else h2t[:, kt - 8, :nt]
                        nc.tensor.matmul(pp[:nt, :], lhsT, Wpost_sb[:, kt, :],
                                         start=(kt == 0), stop=False)
                    nc.tensor.matmul(pp[:nt, :], ones_sb[:, :nt], bpost_sb[:],
                                     start=False, stop=True)
                    nc.scalar.activation(sbatch[:nt, t, :], pp[:nt, :],
                                         mybir.ActivationFunctionType.Sigmoid,
                                         scale=-1.0)
                # softplus(o) = -ln(sigmoid(-o)); one table swap for Ln
                lnb = cpool.tile([128, NT, F], dt.float32)
                nc.scalar.activation(
                    lnb.rearrange("p a b -> p (a b)"),
                    sbatch.rearrange("p a b -> p (a b)"),
                    mybir.ActivationFunctionType.Ln)
                nc.vector.tensor_scalar_mul(
                    lnb.rearrange("p a b -> p (a b)"),
                    lnb.rearrange("p a b -> p (a b)"), -1.0)
                for t in range(NT):
                    nt = min(128, OWN - 128 * t)
                    nc.sync.dma_start(out_ext.ap()[t * 128:t * 128 + nt, :],
                                      lnb[:nt, t, :])
    nc.compile()
    return nc


def _make_runner(nc):
    import jax
    from jax.sharding import Mesh, PartitionSpec
    from jax.experimental.shard_map import shard_map
    import concourse.mybir as mybir
    from concourse.bass2jax import (_bass_exec_p, install_neuronx_cc_hook,
                                    partition_id_tensor)
    install_neuronx_cc_hook()
    partition_name = nc.partition_id_tensor.name if nc.partition_id_tensor else None
    in_names, out_names, out_avals, zero_outs = [], [], [], []
    for alloc in nc.m.functions[0].allocations:
        if not isinstance(alloc, mybir.MemoryLocationSet):
            continue
        name = alloc.memorylocations[0].name
        if alloc.kind == "ExternalInput":
            if name != partition_name:
                in_names.append(name)
        elif alloc.kind == "ExternalOutput":
            out_names.append(name)
            shape = tuple(alloc.tensor_shape)
            dtype = mybir.dt.np(alloc.dtype)
            out_avals.append(jax.core.ShapedArray(shape, dtype))
            zero_outs.append(np.zeros(shape, dtype))
    n_params = len(in_names)
    n_outs = len(out_avals)
    all_in = list(in_names) + list(out_names)
    if partition_name is not None:
        all_in.append(partition_name)
    donate = tuple(range(n_params, n_params + n_outs))

    def _body(*args):
        operands = list(args)
        if partition_name is not None:
            operands.append(partition_id_tensor())
        return tuple(_bass_exec_p.bind(
            *operands, out_avals=tuple(out_avals), in_names=tuple(all_in),
            out_names=tuple(out_names), lowering_input_output_aliases=(),
            sim_require_finite=True, sim_require_nnan=True, nc=nc))

    devices = jax.devices()[:NCORES]
    mesh = Mesh(np.asarray(devices), ("core",))
    fn = jax.jit(
        shard_map(_body, mesh=mesh,
                  in_specs=(PartitionSpec("core"),) * (n_params + n_outs),
                  out_specs=(PartitionSpec("core"),) * n_outs,
                  check_rep=False),
        donate_argnums=donate, keep_unused=True)
    sharding = jax.sharding.NamedSharding(mesh, PartitionSpec("core"))

    def run(in_maps, n_iter=1):
        import time
        gl_ins = [jax.device_put(
            np.concatenate([np.asarray(m[nm]) for m in in_maps], axis=0), sharding)
            for nm in in_names]
        times, outs = [], None
        for _ in range(n_iter):
            gl_zeros = [jax.device_put(np.concatenate([z] * NCORES, axis=0), sharding)
                        for z in zero_outs]
            jax.block_until_ready(gl_zeros)
            t0 = time.perf_counter()
            outs = fn(*gl_ins, *gl_zeros)
            jax.block_until_ready(outs)
            times.append(time.perf_counter() - t0)
        np_outs = [np.asarray(o) for o in outs]
        res = []
        for c in range(NCORES):
            d = {}
            for nm, arr in zip(out_names, np_outs):
                per = arr.shape[0] // NCORES
                d[nm] = arr[c * per:(c + 1) * per]
            res.append(d)
        return res, times
    return run


def _make_in_maps(x, edge_src, edge_dst, W_pre1, b_pre1, W_pre2, b_pre2,
                  W_mp, b_mp, W_post, b_post):
    NL, NH, goff, per_core = _prep_edges(edge_src, edge_dst)
    xb = np.asarray(x, np.float32).astype(bf16)
    common = {
        "W1_in": np.asarray(W_pre1, np.float32).astype(bf16),
        "W2_in": np.asarray(W_pre2, np.float32).astype(bf16),
        "Wmp_in": np.asarray(W_mp, np.float32).astype(bf16),
        "Wpost_in": np.asarray(W_post, np.float32).astype(bf16),
        "b1_in": np.ascontiguousarray(
            np.asarray(b_pre1, np.float32).reshape(8, 128).T),
        "b2_in": np.ascontiguousarray(
            np.asarray(b_pre2, np.float32).reshape(8, 128).T),
        "bmp_in": np.ascontiguousarray(
            np.asarray(b_mp, np.float32).reshape(8, 128).T),
        "bpost_in": np.asarray(b_post, np.float32).astype(bf16).reshape(1, F),
        "ident_in": np.eye(128, dtype=np.float32).astype(bf16),
    }
    in_maps = []
    for c in range(NCORES):
        idx_packed, oh = per_core[c]
        m = dict(common)
        m["xT_in"] = np.ascontiguousarray(xb[c * OWN:(c + 1) * OWN].T)
        m["idx_in"] = idx_packed
        m["oh_in"] = oh
        in_maps.append(m)
    return NL, NH, goff, in_maps


def get_compiled(NL, NH, goff):
    """Build (or fetch cached) compiled kernel for this block schedule."""
    key = (tuple(int(v) for v in NL), tuple(int(v) for v in NH))
    if key not in _cache:
        nc = _build(NL, NH, goff)
        _cache[key] = (nc, _make_runner(nc))
    return _cache[key]


def kernel(**inputs):
    NL, NH, goff, in_maps = _make_in_maps(**inputs)
    _, run = get_compiled(NL, NH, goff)
    res, _ = run(in_maps, n_iter=1)
    return np.concatenate([res[c]["out"] for c in range(NCORES)], axis=0)
